# revision 1
# baseline (speedup 1.0000x reference)
"""Trainium2 Bass kernel for the SAGAN-style self-attention block.

Full-input contract: kernel(**inputs) takes the unsharded numpy inputs and
returns the full-shape output. Internally shards across 8 NeuronCores:
core = (batch_sample, half_of_query_rows).

Math per sample (C=256, Cq=32, N=4096):
    q = (Wq @ F3 + bq) / sqrt(32)        [Cq, N]   (scale folded into q)
    k = Wk @ F1 + bk                     [Cq, N]
    v = Wv @ F2 + bv                     [C, N]
    eT[m, n] = sum_c k[c, m] q[c, n]     (energy, transposed layout)
    E = exp(eT)                          (unnormalized attention, transposed)
    U[c, n] = sum_m v[c, m] E[m, n]
    R[n]    = sum_m E[m, n]              (softmax denominator, ones-row matmul)
    y = gamma * U / R + F3

The transposed-energy layout puts the attention contraction dim (m) on
partitions so the big second matmul needs no transposes; softmax
normalization is deferred past the matmul (exp values are bounded: |e| < ~5).
"""

import numpy as np
import ml_dtypes

N_CORES = 8
B, C, HH, WW = 4, 256, 64, 64
N = HH * WW          # 4096 pixels per sample
CQ = 32              # C // 8 query/key channels
NSH = N // 2         # 2048 query rows per core
NT = 512             # free-dim tile (one PSUM bank of fp32)
MC = 128             # contraction chunk (full partition dim)
ISQ = 1.0 / np.sqrt(32.0)

_BF16 = ml_dtypes.bfloat16
_F8 = ml_dtypes.float8_e4m3
_cache = {}


def _build():
    import concourse.tile as tile
    import concourse.mybir as mybir
    from concourse import bacc
    from contextlib import ExitStack

    f32 = mybir.dt.float32
    bf16 = mybir.dt.bfloat16
    f8 = mybir.dt.float8e4
    Act = mybir.ActivationFunctionType
    from concourse.alu_op_type import AluOpType as Alu

    nc = bacc.Bacc("TRN2", target_bir_lowering=False, debug=False,
                   enable_asserts=False, num_devices=N_CORES)

    x3_d = nc.dram_tensor("x3", [C, NSH], f32, kind="ExternalInput").ap()
    x3b_d = nc.dram_tensor("x3b", [C, NSH], f8, kind="ExternalInput").ap()
    x1_d = nc.dram_tensor("x1", [C, N], f8, kind="ExternalInput").ap()
    x2_d = nc.dram_tensor("x2", [128, 2, N], f8, kind="ExternalInput").ap()
    wv8_d = nc.dram_tensor("wv8", [128, 2, C], f8, kind="ExternalInput").ap()
    # const blobs: cb [128, 640] fp8 = wqt0|wqt1|wkt0|wkt1|wvt0|wvt1
    #              cf [128, 259] f32  = bqs4|bkc4|gam|bvb
    cb_d = nc.dram_tensor("cb", [128, 640], f8, kind="ExternalInput").ap()
    cf_d = nc.dram_tensor("cf", [128, 3 + C], f32, kind="ExternalInput").ap()
    y_d = nc.dram_tensor("y", [C, NSH], f32, kind="ExternalOutput").ap()

    n_mc = N // MC            # 32 contraction chunks
    n_nt = NSH // NT          # 4 query-row tiles per core

    with tile.TileContext(nc) as tc, ExitStack() as ctx:
        const = ctx.enter_context(tc.tile_pool(name="const", bufs=1))
        big = ctx.enter_context(tc.tile_pool(name="big", bufs=1))
        ex_pool = ctx.enter_context(tc.tile_pool(name="ex", bufs=8))
        small = ctx.enter_context(tc.tile_pool(name="small", bufs=2))
        ypool = ctx.enter_context(tc.tile_pool(name="y", bufs=2))

        # ---- constants / weights (two blob DMAs) ----
        cb_sb = const.tile([128, 640], f8, tag="cb", name="cb")
        cf_sb = const.tile([128, 3 + C], f32, tag="cf", name="cf")
        nc.sync.dma_start(cb_sb[:], cb_d[:])
        nc.sync.dma_start(cf_sb[:], cf_d[:])
        wqt_sb = [cb_sb[:, 32 * i:32 * (i + 1)] for i in range(2)]
        wkt_sb = [cb_sb[:, 64 + 32 * i:64 + 32 * (i + 1)] for i in range(2)]
        wvt_sb = [cb_sb[:, 128 + C * i:128 + C * (i + 1)] for i in range(2)]
        bqs_sb = cf_sb[:, 0:1]
        bkc_sb = cf_sb[:, 1:2]
        gam_sb = cf_sb[:, 2:3]
        bvb_sb = cf_sb[:, 3:3 + C]
        bvb2_sb = const.tile([128, 2, C], f32, tag="bvb2", name="bvb2")
        nc.vector.tensor_copy(bvb2_sb[:, 0, :], bvb_sb[:])
        nc.vector.tensor_copy(bvb2_sb[:, 1, :], bvb_sb[:])
        ones_sb = const.tile([128, 2, 128], f8, tag="ones", name="ones")
        nc.vector.memset(ones_sb[:], 1.0)

        # ---- big activations ----
        x3_sb = [big.tile([128, NSH], f32, tag=f"x3_{i}", name=f"x3_{i}") for i in range(2)]
        x3b_sb = [big.tile([128, NSH], f8, tag=f"x3b_{i}", name=f"x3b_{i}") for i in range(2)]
        x1_sb = [big.tile([128, N], f8, tag=f"x1_{i}", name=f"x1_{i}") for i in range(2)]
        x2_sb = big.tile([128, 2, N], f8, tag="x2", name="x2")
        wv8_sb = const.tile([128, 2, C], f8, tag="wv8", name="wv8")
        for i in range(2):
            nc.sync.dma_start(x1_sb[i][:], x1_d[128 * i:128 * (i + 1), :])
        for i in range(2):
            nc.sync.dma_start(x3b_sb[i][:], x3b_d[128 * i:128 * (i + 1), :])
        nc.sync.dma_start(wv8_sb[:], wv8_d[:])
        nc.sync.dma_start(x2_sb[:], x2_d[:])
        # (x3 fp32 residual DMA deferred until after k-projection)

        # q4: q replicated in all 4 partition quadrants [32r+ck, n]
        # k4: chunk jj of k at partition quadrant jj%4, col block jj//4
        q4_sb = big.tile([128, NSH], bf16, tag="q4", name="q4")
        k4_sb = big.tile([128, N // 4], bf16, tag="k4", name="k4")
        vt_sb = big.tile([128, n_mc, C], f8, tag="vt", name="vt")  # [m in chunk, chunk, c]

        psum_e = ctx.enter_context(tc.tile_pool(name="psum_e", bufs=1, space="PSUM"))
        proj_ctx = ExitStack()
        psum_p = proj_ctx.enter_context(
            tc.tile_pool(name="psum_p", bufs=3, space="PSUM"))

        # PE warm-up while input DMAs stream: HAM un-throttles after ~3.4us of
        # sustained matmul activity; these run on the tiny const blob so the
        # real projections start at 2.4 GHz instead of 1.2.
        warm = psum_p.tile([128, NT], f32, tag="pj", name="pj")
        for w in range(8):
            nc.tensor.matmul(warm[:, :C], cb_sb[:, 128:128 + 128],
                             cb_sb[:, 128 + C:128 + C + C],
                             start=True, stop=True)
        nc.vector.tensor_copy(warm[:1, :1], warm[:1, :1])  # keep a reader

        # ---- projections ----
        def kproj(j):
            # k chunks 4j..4j+3 -> quadrant layout via col-group tiling
            kp4 = psum_p.tile([128, MC], f32, tag="pj", name="pj")
            for r in range(4):
                jj = 4 * j + r
                nc.tensor.matmul(kp4[32 * r:32 * (r + 1), :], wkt_sb[0][:],
                                 x1_sb[0][:, MC * jj:MC * (jj + 1)],
                                 start=True, stop=False, tile_position=(0, 32 * r))
                nc.tensor.matmul(kp4[32 * r:32 * (r + 1), :], wkt_sb[1][:],
                                 x1_sb[1][:, MC * jj:MC * (jj + 1)],
                                 start=False, stop=True, tile_position=(0, 32 * r))
            nc.vector.tensor_scalar_add(k4_sb[:, MC * j:MC * (j + 1)], kp4[:],
                                        bkc_sb[:])

        def qproj(j):
            # q n-tile j replicated into all 4 partition quadrants
            qp = psum_p.tile([128, NT], f32, tag="pj", name="pj")
            for r in range(4):
                nc.tensor.matmul(qp[32 * r:32 * (r + 1), :], wqt_sb[0][:],
                                 x3b_sb[0][:, NT * j:NT * (j + 1)],
                                 start=True, stop=False, tile_position=(0, 32 * r))
                nc.tensor.matmul(qp[32 * r:32 * (r + 1), :], wqt_sb[1][:],
                                 x3b_sb[1][:, NT * j:NT * (j + 1)],
                                 start=False, stop=True, tile_position=(0, 32 * r))
            nc.vector.tensor_scalar(q4_sb[:, NT * j:NT * (j + 1)], qp[:],
                                    ISQ, bqs_sb[:], Alu.mult, Alu.add)

        def vtproj(i):
            # vT[m, c] for m-chunk pair (i, i+1): fp8 DoubleRow (K=256 per MM)
            vp = psum_p.tile([128, 2, C], f32, tag="pj", name="pj")
            for u in range(2):
                nc.tensor.matmul(vp[:, u, :], x2_sb[:, :, MC * (i + u):MC * (i + u + 1)],
                                 wv8_sb[:], start=True, stop=True,
                                 perf_mode=mybir.MatmulPerfMode.DoubleRow)
            nc.vector.tensor_add(vt_sb[:, i:i + 2, :], vp[:],
                                 bvb2_sb[:])

        kproj(0)
        qproj(0)

        # ---- attention main loop ----
        # software-pipelined over groups of 4 m-chunks: emit epack(g)+exp(g),
        # then the previous group's U-block (PE covers exp latency).
        # u0/u1/rr accumulators are allocated lazily at each n-tile's first
        # U-block so psum slot reuse follows program order.
        n_grp = n_mc // 4  # 8 groups per n-tile
        groups = [(t, g) for t in range(n_nt) for g in range(n_grp)]
        utiles = {}
        pendq = []

        def emit_group(t, g):
            eps = [psum_e.tile([128, 2, NT], f32, tag=h, name=h) for h in ("pea", "peb")]
            for r in range(4):
                nc.tensor.matmul(eps[r // 2][:, r % 2, :],
                                 k4_sb[32 * r:32 * (r + 1), MC * g:MC * (g + 1)],
                                 q4_sb[32 * r:32 * (r + 1), NT * t:NT * (t + 1)],
                                 start=True, stop=True,
                                 tile_position=(32 * r, 0))
            exs = [ex_pool.tile([128, 2, NT], f8, tag=h, name=h) for h in ("exa", "exb")]
            nc.scalar.activation(exs[0][:], eps[0][:], Act.Exp)
            nc.scalar.activation(exs[1][:], eps[1][:], Act.Exp)
            pendq.append((exs, t, g))

        def u_block(ex4, t, g):
            if g == 0:
                utiles[t] = (
                    psum_a.tile([128, NT], f32, tag="u0", name="u0"),
                    psum_a.tile([128, NT], f32, tag="u1", name="u1"),
                    psum_a.tile([128, NT], f32, tag="rr", name="rr", bufs=2),
                )
            u0, u1, rr = utiles[t]
            DR = mybir.MatmulPerfMode.DoubleRow
            for pr in range(2):
                jj = 4 * g + 2 * pr            # pair covers chunks jj, jj+1
                st, sp = (jj == 0), (jj == n_mc - 2)
                exp_pair = ex4[pr][:]
                nc.tensor.matmul(u0[:], vt_sb[:, jj:jj + 2, 0:128], exp_pair,
                                 start=st, stop=sp, perf_mode=DR)
                nc.tensor.matmul(u1[:], vt_sb[:, jj:jj + 2, 128:C], exp_pair,
                                 start=st, stop=sp, perf_mode=DR)
                nc.tensor.matmul(rr[:], ones_sb[:], exp_pair, start=st, stop=sp,
                                 perf_mode=DR)
            if g == n_grp - 1:
                epilogue(t)

        def epilogue(t):
            # y = gamma * U / R + x3; scale by gamma first to release U psum
            # fast. On the last n-tile the chain is exposed in the kernel tail,
            # so spread it: gamma-scale on ScalarE, c-tile 1 on GpSimd,
            # reciprocal first on DVE.
            last = (t == n_nt - 1)
            u0, u1, rr = utiles.pop(t)
            rec = small.tile([128, NT], f32, tag="rec", name="rec")
            NH = NT // 2
            if last:
                for p in range(2):
                    nc.vector.reciprocal(rec[:, NH * p:NH * (p + 1)],
                                         rr[:, NH * p:NH * (p + 1)])
            t0s = []
            for ct in range(2):
                u = u0 if ct == 0 else u1
                t0 = ypool.tile([128, NT], f32, tag=f"t{ct}", name=f"t{ct}")
                if last:
                    nc.scalar.activation(t0[:], u[:], Act.Identity,
                                         scale=gam_sb[:])
                else:
                    nc.vector.tensor_scalar_mul(t0[:], u[:], gam_sb[:])
                t0s.append(t0)
            for p in range(2):
                if not last:
                    nc.vector.reciprocal(rec[:, NH * p:NH * (p + 1)],
                                         rr[:, NH * p:NH * (p + 1)])
                for ct in range(2):
                    eng = nc.gpsimd if (last and ct == 1) else nc.vector
                    ys = ypool.tile([128, NH], f32, tag=f"ys{ct}", name=f"ys{ct}")
                    eng.tensor_mul(ys[:], t0s[ct][:, NH * p:NH * (p + 1)],
                                   rec[:, NH * p:NH * (p + 1)])
                    ys2 = ypool.tile([128, NH], f32, tag=f"ys2{ct}", name=f"ys2{ct}")
                    eng.tensor_add(
                        ys2[:], ys[:],
                        x3_sb[ct][:, NT * t + NH * p:NT * t + NH * (p + 1)])
                    nc.sync.dma_start(
                        y_d[128 * ct:128 * (ct + 1),
                            NT * t + NH * p:NT * t + NH * (p + 1)], ys2[:])

        # 7 exp groups banked before/during the remaining projections keep
        # ScalarE streaming from ~14us (accumulators not needed yet); kproj(g)
        # is emitted just-in-time before the group that uses it.
        for (t, g) in groups[:7]:
            emit_group(t, g)
            if g + 1 <= 7:
                kproj(g + 1)
        for i in range(0, n_mc, 2):
            vtproj(i)
        for i in range(2):
            nc.vector.tensor_copy(x3_sb[i][:1, :1], k4_sb[:1, MC * 7:MC * 7 + 1])
            nc.sync.dma_start(x3_sb[i][:], x3_d[128 * i:128 * (i + 1), :])
        for j in range(1, 4):
            qproj(j)
        proj_ctx.close()
        psum_a = ctx.enter_context(tc.tile_pool(name="psum_a", bufs=1, space="PSUM"))

        for (t, g) in groups[7:]:
            emit_group(t, g)
            while len(pendq) > 5:
                u_block(*pendq.pop(0))
        while pendq:
            u_block(*pendq.pop(0))

    nc.compile()
    return nc


def _get_nc():
    if "nc" not in _cache:
        _cache["nc"] = _build()
    return _cache["nc"]


def kernel(F3, F1, F2, Wq, bq, Wk, bk, Wv, bv, gamma):
    from concourse import bass_utils

    nc = _get_nc()

    F3 = np.asarray(F3, dtype=np.float32)
    r3 = F3.reshape(B, C, N)
    r1 = np.asarray(F1, dtype=np.float32).reshape(B, C, N)
    r2 = np.asarray(F2, dtype=np.float32).reshape(B, C, N)
    wqt = np.asarray(Wq, np.float32).T
    wkt = np.asarray(Wk, np.float32).T
    wvt = np.asarray(Wv, np.float32).T
    cb = np.empty((128, 640), np.float32)
    cb[:, 0:32] = wqt[:128]; cb[:, 32:64] = wqt[128:]
    cb[:, 64:96] = wkt[:128]; cb[:, 96:128] = wkt[128:]
    cb[:, 128:128 + C] = wvt[:128]; cb[:, 128 + C:] = wvt[128:]
    cb = cb.astype(_F8)
    cf = np.empty((128, 3 + C), np.float32)
    cf[:, 0] = np.tile(np.asarray(bq, np.float32) * ISQ, 4)
    cf[:, 1] = np.tile(np.asarray(bk, np.float32), 4)
    cf[:, 2] = np.float32(np.asarray(gamma).reshape(()))
    cf[:, 3:] = np.asarray(bv, np.float32)[None, :]
    wv8 = np.ascontiguousarray(
        np.asarray(Wv, np.float32).T.reshape(2, 128, C).transpose(1, 0, 2)).astype(_F8)
    in_maps = []
    for cid in range(N_CORES):
        b, h = divmod(cid, 2)
        in_maps.append({
            "x3": np.ascontiguousarray(r3[b][:, NSH * h:NSH * (h + 1)]),
            "x3b": np.ascontiguousarray(r3[b][:, NSH * h:NSH * (h + 1)]).astype(_F8),
            "x1": r1[b].astype(_F8),
            "x2": np.ascontiguousarray(
                r2[b].reshape(2, 128, N).transpose(1, 0, 2)).astype(_F8),
            "wv8": wv8,
            "cb": cb, "cf": cf,
        })

    _cache["in_maps"] = in_maps
    res = bass_utils.run_bass_kernel_spmd(nc, in_maps, core_ids=list(range(N_CORES)))
    out = np.empty((B, C, N), np.float32)
    for cid in range(N_CORES):
        b, h = divmod(cid, 2)
        out[b][:, NSH * h:NSH * (h + 1)] = res.results[cid]["y"]
    return out.reshape(B, C, HH, WW)



# revision 11
# speedup vs baseline: 1.0031x; 1.0031x over previous
"""Trainium2 Bass kernel for the SAGAN-style self-attention block.

Full-input contract: kernel(**inputs) takes the unsharded numpy inputs and
returns the full-shape output. Internally shards across 8 NeuronCores:
core = (batch_sample, half_of_query_rows).

Math per sample (C=256, Cq=32, N=4096):
    q = (Wq @ F3 + bq) / sqrt(32)        [Cq, N]   (scale folded into q)
    k = Wk @ F1 + bk                     [Cq, N]
    v0 = Wv @ F2                         [C, N]    (bias folded: see below)
    eT[m, n] = sum_c k[c, m] q[c, n]     (energy, transposed layout)
    E = exp(eT)                          (unnormalized attention, transposed)
    U[c, n] = sum_m v0[c, m] E[m, n]
    R[n]    = sum_m E[m, n]              (softmax denominator, ones-row matmul)
    y = gamma * U / R + (F3 + gamma*bv)  (bv folded: U_full = U + bv*R)

Pipeline structure: the energy PSUM is double-buffered (2 m-chunks per
group, [128,2,512] = 2 banks, bufs=2) so the PE's energy matmuls for
group g+1 overlap ScalarE's exp of group g; u-block matmuls (DoubleRow
fp8) fill the remaining PE slots. PSUM: 4 (energy) + 1 (qproj) + 3
(u0/u1/rr) = 8 banks.
"""

import numpy as np
import ml_dtypes

N_CORES = 8
B, C, HH, WW = 4, 256, 64, 64
N = HH * WW          # 4096 pixels per sample
CQ = 32              # C // 8 query/key channels
NSH = N // 2         # 2048 query rows per core
NT = 512             # free-dim tile (one n-tile)
MC = 128             # contraction chunk (full partition dim)
ISQ = 1.0 / np.sqrt(32.0)

_BF16 = ml_dtypes.bfloat16
_F8 = ml_dtypes.float8_e4m3
_cache = {}


def _build():
    import concourse.tile as tile
    import concourse.mybir as mybir
    from concourse import bacc
    from contextlib import ExitStack
    from collections import deque

    f32 = mybir.dt.float32
    bf16 = mybir.dt.bfloat16
    f8 = mybir.dt.float8e4
    Act = mybir.ActivationFunctionType
    DR = mybir.MatmulPerfMode.DoubleRow
    from concourse.alu_op_type import AluOpType as Alu

    nc = bacc.Bacc("TRN2", target_bir_lowering=False, debug=False,
                   enable_asserts=False, num_devices=N_CORES)

    x3_d = nc.dram_tensor("x3", [C, NSH], f32, kind="ExternalInput").ap()
    x3b_d = nc.dram_tensor("x3b", [128, 2, NSH], f8, kind="ExternalInput").ap()
    x1_d = nc.dram_tensor("x1", [128, 2, N], f8, kind="ExternalInput").ap()
    x2_d = nc.dram_tensor("x2", [128, 2, N], f8, kind="ExternalInput").ap()
    wv8_d = nc.dram_tensor("wv8", [128, 2, C], f8, kind="ExternalInput").ap()
    cb_d = nc.dram_tensor("cb", [128, 2, 64], f8, kind="ExternalInput").ap()
    cf_d = nc.dram_tensor("cf", [128, 3], f32, kind="ExternalInput").ap()
    y_d = nc.dram_tensor("y", [C, NSH], f32, kind="ExternalOutput").ap()

    n_nt = NSH // NT          # 4 query-row tiles per core
    n_pairs = N // (2 * MC)   # 16 m-chunk pairs
    n_grp = n_nt * n_pairs    # 64 groups, one [128,2,NT] energy psum each

    with tile.TileContext(nc) as tc, ExitStack() as ctx:
        const = ctx.enter_context(tc.tile_pool(name="const", bufs=1))
        big = ctx.enter_context(tc.tile_pool(name="big", bufs=1))
        ex_pool = ctx.enter_context(tc.tile_pool(name="ex", bufs=16))
        small = ctx.enter_context(tc.tile_pool(name="small", bufs=2))
        ypool = ctx.enter_context(tc.tile_pool(name="y", bufs=2))

        # ---- local constants (no DMA dependency) ----
        warm_sb = const.tile([128, 256], f8, tag="warm", name="warm")
        nc.vector.memset(warm_sb[:], 0.0625)
        ones_sb = const.tile([128, 2, 128], f8, tag="ones", name="ones")
        nc.vector.memset(ones_sb[:], 1.0)

        # ---- weights / activations ----
        cb_sb = const.tile([128, 2, 64], f8, tag="cb", name="cb")
        cf_sb = const.tile([128, 3], f32, tag="cf", name="cf")
        wv8_sb = const.tile([128, 2, C], f8, tag="wv8", name="wv8")
        x1_sb = big.tile([128, 2, N], f8, tag="x1", name="x1")
        x3b_sb = big.tile([128, 2, NSH], f8, tag="x3b", name="x3b")
        x2_sb = big.tile([128, 2, N], f8, tag="x2", name="x2")
        x3_sb = [big.tile([128, NSH], f32, tag=f"x3_{i}", name=f"x3_{i}")
                 for i in range(2)]

        wq_sb = cb_sb[:, :, 0:32]
        wk_sb = cb_sb[:, :, 32:64]
        bqs_sb = cf_sb[:, 0:1]
        bkc_sb = cf_sb[:, 1:2]
        gam_sb = cf_sb[:, 2:3]

        # input DMA stream, priority order (sync queue)
        nc.sync.dma_start(x1_sb[:, :, 0:1024], x1_d[:, :, 0:1024])
        nc.sync.dma_start(cb_sb[:], cb_d[:])
        nc.sync.dma_start(cf_sb[:], cf_d[:])
        nc.sync.dma_start(x3b_sb[:, :, 0:512], x3b_d[:, :, 0:512])
        nc.sync.dma_start(x1_sb[:, :, 1024:2048], x1_d[:, :, 1024:2048])
        nc.sync.dma_start(x3b_sb[:, :, 512:1024], x3b_d[:, :, 512:1024])
        nc.sync.dma_start(x2_sb[:, :, 0:2048], x2_d[:, :, 0:2048])
        nc.sync.dma_start(x1_sb[:, :, 2048:3072], x1_d[:, :, 2048:3072])
        nc.sync.dma_start(wv8_sb[:], wv8_d[:])
        nc.sync.dma_start(x1_sb[:, :, 3072:4096], x1_d[:, :, 3072:4096])
        nc.sync.dma_start(x2_sb[:, :, 2048:4096], x2_d[:, :, 2048:4096])
        nc.sync.dma_start(x3b_sb[:, :, 1024:1536], x3b_d[:, :, 1024:1536])
        nc.sync.dma_start(x3b_sb[:, :, 1536:2048], x3b_d[:, :, 1536:2048])
        # (x3 fp32 residual DMA is issued mid-schedule from the gpsimd queue,
        # gated behind the k projection, so its 2MB doesn't compete with the
        # startup-critical DMAs)

        # q4: q replicated in all 4 partition quadrants [32r+ck, n]
        # k4: chunk jj of k at partition quadrant jj%4, col block jj//4
        q4_sb = big.tile([128, NSH], bf16, tag="q4", name="q4")
        k4_sb = big.tile([128, N // 4], bf16, tag="k4", name="k4")
        vt_sb = big.tile([128, 2 * n_pairs, C], f8, tag="vt", name="vt")

        psum_e = ctx.enter_context(
            tc.tile_pool(name="psum_e", bufs=2, space="PSUM"))
        proj_ctx = ExitStack()
        psum_p = proj_ctx.enter_context(
            tc.tile_pool(name="psum_p", bufs=3, space="PSUM"))
        qpool = [psum_p]  # swapped to psum_a for the late q projections

        # PE warm-up on the memset tile (no DMA dependency): HAM un-throttles
        # after sustained matmul activity, so the real projections start fast.
        warmp = psum_p.tile([128, NT], f32, tag="pj", name="pj")
        for w in range(8):
            nc.tensor.matmul(warmp[:, 0:256], warm_sb[:, 0:128], warm_sb[:],
                             start=True, stop=True)
        nc.vector.tensor_copy(warmp[:1, :1], warmp[:1, :1])  # keep a reader

        # ---- projections ----
        # (DoubleRow needs dst partition 0, so the quadrant-offset k/q
        # projections use paired plain fp8 matmuls over the c-halves)
        def kproj(j):
            # k chunks 4j..4j+3 -> quadrant layout via col-group tiling
            kp = psum_p.tile([128, MC], f32, tag="pj", name="pj")
            for r in range(4):
                jj = 4 * j + r
                for i in range(2):
                    nc.tensor.matmul(kp[32 * r:32 * (r + 1), :],
                                     cb_sb[:, i, 32:64],
                                     x1_sb[:, i, MC * jj:MC * (jj + 1)],
                                     start=(i == 0), stop=(i == 1),
                                     tile_position=(0, 32 * r))
            nc.vector.tensor_scalar_add(k4_sb[:, MC * j:MC * (j + 1)], kp[:],
                                        bkc_sb[:])

        def qproj(t):
            # q n-tile t replicated into all 4 partition quadrants
            qp = qpool[0].tile([128, NT], f32, tag="qj", name="qj", bufs=1)
            for r in range(4):
                for i in range(2):
                    nc.tensor.matmul(qp[32 * r:32 * (r + 1), :],
                                     cb_sb[:, i, 0:32],
                                     x3b_sb[:, i, NT * t:NT * (t + 1)],
                                     start=(i == 0), stop=(i == 1),
                                     tile_position=(0, 32 * r))
            nc.vector.tensor_scalar(q4_sb[:, NT * t:NT * (t + 1)], qp[:],
                                    ISQ, bqs_sb[:], Alu.mult, Alu.add)

        def vtproj(i):
            # vT[m, c] for m-chunk pair (2i, 2i+1): fp8 DoubleRow (K=256)
            vp = psum_p.tile([128, 2, C], f32, tag="pj", name="pj")
            for u in range(2):
                jj = 2 * i + u
                nc.tensor.matmul(vp[:, u, :],
                                 x2_sb[:, :, MC * jj:MC * (jj + 1)],
                                 wv8_sb[:], start=True, stop=True,
                                 perf_mode=DR)
            nc.vector.tensor_copy(vt_sb[:, 2 * i:2 * i + 2, :], vp[:])

        # ---- attention main loop ----
        pend = deque()
        utiles = {}

        def egroup(g):
            t, p = divmod(g, n_pairs)
            ep = psum_e.tile([128, 2, NT], f32, tag="ep", name="ep")
            for r in range(2):
                jj = 2 * p + r
                quad, blk = jj % 4, jj // 4
                nc.tensor.matmul(ep[:, r, :],
                                 k4_sb[32 * quad:32 * (quad + 1),
                                       MC * blk:MC * (blk + 1)],
                                 q4_sb[32 * quad:32 * (quad + 1),
                                       NT * t:NT * (t + 1)],
                                 start=True, stop=True,
                                 tile_position=(32 * quad, 0))
            ex = ex_pool.tile([128, 2, NT], f8, tag="ex", name="ex")
            nc.scalar.activation(ex[:], ep[:], Act.Exp)
            pend.append((ex, g))

        def ublock(ex, g):
            t, p = divmod(g, n_pairs)
            if p == 0:
                utiles[t] = (
                    psum_a.tile([128, NT], f32, tag="u0", name="u0"),
                    psum_a.tile([128, NT], f32, tag="u1", name="u1"),
                    psum_a.tile([128, NT], f32, tag="rr", name="rr"),
                )
            u0, u1, rr = utiles[t]
            st, sp = (p == 0), (p == n_pairs - 1)
            vpair = vt_sb[:, 2 * p:2 * p + 2, :]
            if sp:
                # rr finishes first so the reciprocal can start earliest
                nc.tensor.matmul(rr[:], ones_sb[:], ex[:], start=st, stop=sp,
                                 perf_mode=DR)
            nc.tensor.matmul(u0[:], vpair[:, :, 0:128], ex[:], start=st,
                             stop=sp, perf_mode=DR)
            nc.tensor.matmul(u1[:], vpair[:, :, 128:C], ex[:], start=st,
                             stop=sp, perf_mode=DR)
            if not sp:
                nc.tensor.matmul(rr[:], ones_sb[:], ex[:], start=st, stop=sp,
                                 perf_mode=DR)
            if sp:
                epilogue(t)

        def epilogue(t):
            # y = gamma * U / R + x3'   (x3' has gamma*bv folded in)
            u0, u1, rr = utiles.pop(t)
            last = (t == n_nt - 1)
            rec = small.tile([128, NT], f32, tag="rec", name="rec")
            nc.vector.reciprocal_approx_fast(rec[:], rr[:])
            cl = slice(NT * t, NT * (t + 1))
            if not last:
                # c-half 0 on DVE, c-half 1 on GpSimd (idle engine)
                ys0 = ypool.tile([128, NT], f32, tag="ys0", name="ys0")
                nc.vector.scalar_tensor_tensor(ys0[:], rec[:], gam_sb[:],
                                               u0[:], Alu.mult, Alu.mult)
                yo0 = ypool.tile([128, NT], f32, tag="yo0", name="yo0")
                nc.vector.tensor_add(yo0[:], ys0[:], x3_sb[0][:, cl])
                nc.sync.dma_start(y_d[0:128, cl], yo0[:])
                t1 = ypool.tile([128, NT], f32, tag="t1", name="t1")
                nc.vector.tensor_scalar_mul(t1[:], u1[:], gam_sb[:])
                ys1 = ypool.tile([128, NT], f32, tag="ys1", name="ys1")
                nc.gpsimd.tensor_mul(ys1[:], t1[:], rec[:])
                yo1 = ypool.tile([128, NT], f32, tag="yo1", name="yo1")
                nc.gpsimd.tensor_add(yo1[:], ys1[:], x3_sb[1][:, cl])
                nc.gpsimd.dma_start(y_d[128:C, cl], yo1[:])
            else:
                # tail: split into quarters, 3 on DVE + 1 on GpSimd (which
                # cannot read PSUM, so DVE spills gamma*u1 quarter to SBUF)
                NH = NT // 2
                qs3 = slice(NH, NT)
                t1q = ypool.tile([128, NH], f32, tag="t1q", name="t1q")
                nc.vector.tensor_scalar_mul(t1q[:], u1[:, qs3], gam_sb[:])
                parts = [(0, 0), (0, 1), (1, 0)]
                for ct, q in parts:
                    u = u0 if ct == 0 else u1
                    qs = slice(NH * q, NH * (q + 1))
                    gs = slice(NT * t + NH * q, NT * t + NH * (q + 1))
                    ys = ypool.tile([128, NH], f32, tag=f"lys{ct}{q}",
                                    name=f"lys{ct}{q}")
                    nc.vector.scalar_tensor_tensor(ys[:], rec[:, qs],
                                                   gam_sb[:], u[:, qs],
                                                   Alu.mult, Alu.mult)
                    yo = ypool.tile([128, NH], f32, tag=f"lyo{ct}{q}",
                                    name=f"lyo{ct}{q}")
                    nc.vector.tensor_add(yo[:], ys[:], x3_sb[ct][:, gs])
                    nc.sync.dma_start(y_d[128 * ct:128 * (ct + 1), gs], yo[:])
                gs3 = slice(NT * t + NH, NT * (t + 1))
                ys3 = ypool.tile([128, NH], f32, tag="lys11", name="lys11")
                nc.gpsimd.tensor_mul(ys3[:], t1q[:], rec[:, qs3])
                yo3 = ypool.tile([128, NH], f32, tag="lyo11", name="lyo11")
                nc.gpsimd.tensor_add(yo3[:], ys3[:], x3_sb[1][:, gs3])
                nc.gpsimd.dma_start(y_d[128:C, gs3], yo3[:])

        # ---- emission schedule ----
        # proj phase: interleave projections with energy groups 0..13 so
        # ScalarE starts streaming exp immediately.
        kproj(0)
        qproj(0)
        egroup(0)
        kproj(1)
        egroup(1)
        kproj(2)
        egroup(2)
        kproj(3)
        vtproj(0)
        egroup(3)
        qproj(1)
        egroup(4)
        kproj(4)
        vtproj(1)
        egroup(5)
        kproj(5)
        vtproj(2)
        egroup(6)
        kproj(6)
        vtproj(3)
        egroup(7)
        kproj(7)
        vtproj(4)
        egroup(8)
        # release the x3 residual DMAs now that k is mostly projected: the
        # gating copy makes the gpsimd queue wait for kproj(4)'s output.
        nc.gpsimd.tensor_copy(x3_sb[0][:1, :1], k4_sb[:1, MC * 4:MC * 4 + 1])
        nc.gpsimd.dma_start(x3_sb[0][:], x3_d[0:128, :])
        nc.gpsimd.dma_start(x3_sb[1][:], x3_d[128:C, :])
        vtproj(5)
        vtproj(6)
        egroup(9)
        vtproj(7)
        vtproj(8)
        egroup(10)
        vtproj(9)
        vtproj(10)
        egroup(11)
        vtproj(11)
        vtproj(12)
        egroup(12)
        vtproj(13)
        vtproj(14)
        egroup(13)
        vtproj(15)
        proj_ctx.close()
        psum_a = ctx.enter_context(
            tc.tile_pool(name="psum_a", bufs=1, space="PSUM"))
        qpool[0] = psum_a

        # steady state: 1 energy group per u-block, draining the banked
        # backlog (14 groups) to 2 with periodic extra u-blocks.
        extra = 12
        for g in range(14, n_grp):
            if g == 18:
                qproj(2)
            if g == 34:
                qproj(3)
            egroup(g)
            ublock(*pend.popleft())
            if extra > 0 and g % 4 == 1:
                ublock(*pend.popleft())
                extra -= 1
        while pend:
            ublock(*pend.popleft())

    nc.compile()
    return nc


def _get_nc():
    if "nc" not in _cache:
        _cache["nc"] = _build()
    return _cache["nc"]


def kernel(F3, F1, F2, Wq, bq, Wk, bk, Wv, bv, gamma):
    from concourse import bass_utils

    nc = _get_nc()

    F3 = np.asarray(F3, dtype=np.float32)
    r3 = F3.reshape(B, C, N)
    r1 = np.asarray(F1, dtype=np.float32).reshape(B, C, N)
    r2 = np.asarray(F2, dtype=np.float32).reshape(B, C, N)
    gam = float(np.asarray(gamma, np.float32).reshape(()))

    def _dr(w):  # [O, C] -> interleaved [128, 2, O] fp8
        return np.ascontiguousarray(
            np.asarray(w, np.float32).T.reshape(2, 128, -1).transpose(1, 0, 2))

    cb = np.concatenate([_dr(Wq), _dr(Wk)], axis=2).astype(_F8)  # [128,2,64]
    cf = np.empty((128, 3), np.float32)
    cf[:, 0] = np.tile(np.asarray(bq, np.float32) * ISQ, 4)
    cf[:, 1] = np.tile(np.asarray(bk, np.float32), 4)
    cf[:, 2] = gam
    wv8 = _dr(Wv).astype(_F8)                                    # [128,2,256]
    bvg = gam * np.asarray(bv, np.float32)                       # [C]

    in_maps = []
    for cid in range(N_CORES):
        b, h = divmod(cid, 2)
        x3h = r3[b][:, NSH * h:NSH * (h + 1)]
        in_maps.append({
            "x3": np.ascontiguousarray(x3h + bvg[:, None]),
            "x3b": np.ascontiguousarray(
                x3h.reshape(2, 128, NSH).transpose(1, 0, 2)).astype(_F8),
            "x1": np.ascontiguousarray(
                r1[b].reshape(2, 128, N).transpose(1, 0, 2)).astype(_F8),
            "x2": np.ascontiguousarray(
                r2[b].reshape(2, 128, N).transpose(1, 0, 2)).astype(_F8),
            "wv8": wv8,
            "cb": cb, "cf": cf,
        })

    _cache["in_maps"] = in_maps
    res = bass_utils.run_bass_kernel_spmd(nc, in_maps, core_ids=list(range(N_CORES)))
    out = np.empty((B, C, N), np.float32)
    for cid in range(N_CORES):
        b, h = divmod(cid, 2)
        out[b][:, NSH * h:NSH * (h + 1)] = res.results[cid]["y"]
    return out.reshape(B, C, HH, WW)
